# revision 12
# baseline (speedup 1.0000x reference)
"""Trainium2 Bass kernel for a single-step DecoderRNN (embed+ReLU -> GRU -> logits -> log_softmax).

Sharding across 8 NeuronCores:
  - GRU weights are E-sharded: core i computes h_new[i*128:(i+1)*128] (one
    128-slice of each of the r/z/n gates), then an AllGather assembles the
    full h_new on every core.
  - W_out / b_out are vocab-sharded (row-parallel): core i computes logits
    for its padded 6400-row vocab shard, plus a local sum(exp(logits)).
    An AllGather of the 8 partial sums gives every core the global
    log-sum-exp, which is subtracted locally (no max-subtraction needed:
    logits are O(1) for this model so exp() cannot overflow).

The dominant cost is streaming the W_out shard (25.6 MB/core) from HBM, so
the kernel is organized as a chunk-major stream: for each 512-column chunk
of the vocab shard the weights arrive as one DMA and are immediately
consumed by 8 (+1 bias) matmuls, with the GRU + collective latency hidden
under the first few chunk DMAs.
"""

import sys

sys.path.insert(0, "/opt/trn_rl_repo")

import numpy as np

from concourse import bacc, bass, mybir, tile
from concourse.bass_utils import run_bass_kernel_spmd

F32 = mybir.dt.float32
F32R = mybir.dt.float32r

VOCAB = 50257
EMB = 1024
NCORES = 8
KT = EMB // 128           # 8 k-tiles over the embedding dim
CHUNKS = [512] * 12 + [256]
VPC = sum(CHUNKS)         # 6400 padded vocab rows per core
VPAD = VPC * NCORES       # 51200
BPAD = -87.0              # bias value for padded vocab rows: exp(-87) ~= 0

# matmul dtype for the big output-projection matvec: float32r streams the
# moving operand at 1 cycle/row (vs 4 for float32) with fp32-class accuracy.
MM_DTYPE = "fp32r"


def build_nc(ncores=NCORES, kt=KT, chunks=CHUNKS, mm_dtype=MM_DTYPE):
    """Build the SPMD Bass program (identical on all cores; data differs)."""
    emb = kt * 128
    vpc = sum(chunks)
    assert ncores == kt, "E-shard per core must be exactly 128 rows"
    nc = bacc.Bacc(
        "TRN2", target_bir_lowering=False, debug=False, num_devices=ncores
    )

    # The big-matvec operands are declared float32r when mm_dtype == "fp32r"
    # (1 cycle/row on the PE vs 4 for float32): the BIR verifier requires
    # every producer of an fp32r-matmult operand to be typed float32r, so the
    # whole dataflow (DRAM input -> DMA -> SBUF tile -> matmul) uses wdt.
    wdt = F32R if mm_dtype == "fp32r" else F32

    def mk(ap):
        return ap.bitcast(wdt) if ap.dtype != wdt else ap

    # ---- kernel I/O (per core) ----
    # xh: interleaved grids, col 2k = x[k*128:(k+1)*128] (pre-ReLU emb row),
    #     col 2k+1 = h[k*128:(k+1)*128]
    xh_d = nc.dram_tensor("xh", [128, 2 * kt], F32, kind="ExternalInput").ap()
    hcol_d = nc.dram_tensor("hcol", [128, 1], F32, kind="ExternalInput").ap()
    wgru_d = nc.dram_tensor(
        "w_gru", [128, 6 * kt * 128], F32, kind="ExternalInput"
    ).ap()  # blocks j=(gate*2+mat)*kt+k at cols j*128..(j+1)*128
    bstk_d = nc.dram_tensor("b_stack", [2, 384], F32, kind="ExternalInput").ap()
    cst_d = nc.dram_tensor("cst", [2, 4], F32, kind="ExternalInput").ap()
    wout_d = nc.dram_tensor(
        "w_out", [128, kt * vpc], wdt, kind="ExternalInput"
    ).ap()
    bout_d = nc.dram_tensor("b_out", [1, vpc], wdt, kind="ExternalInput").ap()
    onesr_d = nc.dram_tensor("onesr", [1, 1], wdt, kind="ExternalInput").ap()

    out_lp = nc.dram_tensor("out_logp", [1, vpc], F32, kind="ExternalOutput").ap()
    out_h = nc.dram_tensor("out_h", [1, emb], F32, kind="ExternalOutput").ap()

    groups = [list(range(ncores))]

    with tile.TileContext(nc) as tc:
        with (
            tc.tile_pool(name="consts", bufs=1) as cpool,
            tc.tile_pool(name="wgru", bufs=1) as gpool,
            tc.tile_pool(name="wout", bufs=6) as wpool,
            tc.tile_pool(name="work", bufs=1) as spool,
            tc.tile_pool(name="esc", bufs=2) as epool,
            tc.tile_pool(name="gps", bufs=2, space="PSUM") as pgpool,
            tc.tile_pool(name="gpn", bufs=1, space="PSUM") as pnpool,
            tc.tile_pool(name="cps", bufs=5, space="PSUM") as pcpool,
            tc.tile_pool(name="dram", bufs=1, space="DRAM") as dpool,
        ):
            # ---- small input loads ----
            xh = cpool.tile([128, 2 * kt], F32, tag="xh")
            nc.sync.dma_start(out=xh[:, :], in_=xh_d[:, :])
            hcol = cpool.tile([128, 1], F32, tag="hcol")
            nc.sync.dma_start(out=hcol[:, :], in_=hcol_d[:, :])
            bstk = cpool.tile([2, 384], F32, tag="bstk")
            nc.sync.dma_start(out=bstk[:, :], in_=bstk_d[:, :])
            cst = cpool.tile([2, 4], F32, tag="cst")
            nc.sync.dma_start(out=cst[:, :], in_=cst_d[:, :])
            bout = cpool.tile([1, vpc], wdt, tag="bout")
            nc.sync.dma_start(out=bout[:, :], in_=bout_d[:, :])
            onesr = cpool.tile([1, 1], wdt, tag="onesr")
            nc.sync.dma_start(out=onesr[:, :], in_=onesr_d[:, :])

            # GRU weights: one tile per (gate, matrix) piece so each matmul
            # depends on a single DMA (HW limits sync-waits per instruction)
            # and the six loads spread across DMA queues.
            piece = kt * 128
            wg = []
            for q in range(6):
                t = gpool.tile([128, piece], F32, tag=f"wg{q}")
                nc.sync.dma_start(
                    out=t[:, :], in_=wgru_d[:, q * piece:(q + 1) * piece]
                )
                wg.append(t)

            # ---- ReLU on the x columns (even columns of xh) ----
            nc.scalar.activation(
                xh[:, 0:2 * kt:2], xh[:, 0:2 * kt:2],
                mybir.ActivationFunctionType.Relu,
            )

            # ---- GRU gate matvecs ----
            # gate g (0=r, 1=z, 2=n), mat m (0=ih, 1=hh):
            #   block j = (g*2 + m)*kt + k  at wgru cols j*128..(j+1)*128
            ones2 = cst[0:2, 0:1]
            i2c = [cst[0:2, 1:2], cst[0:2, 2:3]]

            def gate_mm(ps_out, g, m, first, skip=False):
                for k in range(kt):
                    nc.tensor.matmul(
                        ps_out,
                        lhsT=wg[g * 2 + m][:, k * 128:(k + 1) * 128],
                        rhs=xh[:, 2 * k + m:2 * k + m + 1],
                        start=(first and k == 0),
                        stop=False,
                        skip_group_check=skip,
                    )

            # r and z: accumulate ih+hh+both biases into one [128,1] psum
            ps_r = pgpool.tile([128, 1], F32, tag="ps_rz")
            gate_mm(ps_r[:, :], 0, 0, True)
            gate_mm(ps_r[:, :], 0, 1, False)
            nc.tensor.matmul(
                ps_r[:, :], lhsT=bstk[:, 0:128], rhs=ones2,
                start=False, stop=True,
            )
            ps_z = pgpool.tile([128, 1], F32, tag="ps_rz")
            gate_mm(ps_z[:, :], 1, 0, True)
            gate_mm(ps_z[:, :], 1, 1, False)
            nc.tensor.matmul(
                ps_z[:, :], lhsT=bstk[:, 128:256], rhs=ones2,
                start=False, stop=True,
            )
            # n: keep ih / hh separate in two psum columns
            ps_n = pnpool.tile([128, 2], F32, tag="ps_n")
            gate_mm(ps_n[:, 0:1], 2, 0, True, skip=True)
            nc.tensor.matmul(
                ps_n[:, 0:1], lhsT=bstk[:, 256:384], rhs=i2c[0],
                start=False, stop=True, skip_group_check=True,
            )
            gate_mm(ps_n[:, 1:2], 2, 1, True, skip=True)
            nc.tensor.matmul(
                ps_n[:, 1:2], lhsT=bstk[:, 256:384], rhs=i2c[1],
                start=False, stop=True, skip_group_check=True,
            )

            # ---- GRU elementwise (all [128,1], partition-parallel) ----
            r_sb = spool.tile([128, 1], F32, tag="r")
            nc.scalar.activation(
                r_sb[:, :], ps_r[:, :], mybir.ActivationFunctionType.Sigmoid
            )
            z_sb = spool.tile([128, 1], F32, tag="z")
            nc.scalar.activation(
                z_sb[:, :], ps_z[:, :], mybir.ActivationFunctionType.Sigmoid
            )
            n2 = spool.tile([128, 2], F32, tag="n2")
            nc.vector.tensor_copy(n2[:, :], ps_n[:, :])
            npre = spool.tile([128, 1], F32, tag="npre")
            # npre = (h_n * r) + i_n
            nc.vector.scalar_tensor_tensor(
                npre[:, :], n2[:, 1:2], r_sb[:, :], n2[:, 0:1],
                op0=mybir.AluOpType.mult, op1=mybir.AluOpType.add,
            )
            n_sb = spool.tile([128, 1], F32, tag="n")
            nc.scalar.activation(
                n_sb[:, :], npre[:, :], mybir.ActivationFunctionType.Tanh
            )
            d_sb = spool.tile([128, 1], F32, tag="d")
            nc.vector.tensor_sub(d_sb[:, :], hcol[:, :], n_sb[:, :])
            hnew = spool.tile([128, 1], F32, tag="hnew")
            # h_new = (h - n) * z + n
            nc.vector.scalar_tensor_tensor(
                hnew[:, :], d_sb[:, :], z_sb[:, :], n_sb[:, :],
                op0=mybir.AluOpType.mult, op1=mybir.AluOpType.add,
            )

            # ---- AllGather h_new slices -> full h_new on every core ----
            cc_in = dpool.tile([128, 1], F32, tag="cc_in")
            cc_out = dpool.tile([ncores, 128], F32, tag="cc_out")
            nc.gpsimd.dma_start(out=cc_in[:, :], in_=hnew[:, :])
            nc.gpsimd.collective_compute(
                "AllGather",
                mybir.AluOpType.bypass,
                replica_groups=groups,
                ins=[cc_in.opt()],
                outs=[cc_out.opt()],
            )
            # full h_new, grid layout [128, kt] (col k = h_new[k*128:(k+1)*128])
            hg = spool.tile([128, kt], wdt, tag="hg")
            nc.gpsimd.dma_start(
                out=hg[:, 0:kt], in_=mk(cc_out[:, :].transpose([1, 0]))
            )
            # also emit the h_new kernel output (layouts match linearly)
            nc.gpsimd.dma_start(out=out_h[0:1, 0:emb], in_=cc_out[:, :])

            # ---- output projection: logits chunks + exp-sums ----
            logits = spool.tile([1, vpc], F32, tag="logits")
            sums = spool.tile([1, 16], F32, tag="sums")
            off = 0
            for c, ch in enumerate(chunks):
                wt = wpool.tile([128, kt * 512], wdt, tag="wt")
                nc.sync.dma_start(
                    out=wt[:, 0:kt * ch],
                    in_=wout_d[:, kt * off:kt * (off + ch)],
                )
                ps = pcpool.tile([1, 512], F32, tag="cps")
                for k in range(kt):
                    nc.tensor.matmul(
                        ps[0:1, 0:ch],
                        lhsT=hg[:, k:k + 1],
                        rhs=wt[:, k * ch:(k + 1) * ch],
                        start=(k == 0),
                        stop=False,
                    )
                # fold in b_out via a K=1 matmul with a ones stationary
                nc.tensor.matmul(
                    ps[0:1, 0:ch],
                    lhsT=onesr[0:1, 0:1],
                    rhs=bout[0:1, off:off + ch],
                    start=False,
                    stop=True,
                )
                nc.vector.tensor_copy(logits[0:1, off:off + ch], ps[0:1, 0:ch])
                esc = epool.tile([1, 512], F32, tag="esc")
                nc.scalar.activation(
                    esc[0:1, 0:ch],
                    ps[0:1, 0:ch],
                    mybir.ActivationFunctionType.Exp,
                    accum_out=sums[0:1, c:c + 1],
                )
                off += ch

            # ---- global log-sum-exp via AllGather of the 8 partial sums ----
            sloc = spool.tile([1, 1], F32, tag="sloc")
            nc.vector.tensor_reduce(
                sloc[:, :], sums[0:1, 0:len(chunks)],
                axis=mybir.AxisListType.X, op=mybir.AluOpType.add,
            )
            cc2_in = dpool.tile([1, 1], F32, tag="cc2_in")
            cc2_out = dpool.tile([ncores, 1], F32, tag="cc2_out")
            nc.gpsimd.dma_start(out=cc2_in[:, :], in_=sloc[:, :])
            nc.gpsimd.collective_compute(
                "AllGather",
                mybir.AluOpType.bypass,
                replica_groups=groups,
                ins=[cc2_in.opt()],
                outs=[cc2_out.opt()],
            )
            s8 = spool.tile([1, ncores], F32, tag="s8")
            nc.gpsimd.dma_start(out=s8[0:1, 0:ncores], in_=cc2_out[:, :])
            stot = spool.tile([1, 1], F32, tag="stot")
            nc.vector.tensor_reduce(
                stot[:, :], s8[0:1, 0:ncores],
                axis=mybir.AxisListType.X, op=mybir.AluOpType.add,
            )
            lse = spool.tile([1, 1], F32, tag="lse")
            nc.scalar.activation(
                lse[:, :], stot[:, :], mybir.ActivationFunctionType.Ln
            )
            nlse = spool.tile([1, 1], F32, tag="nlse")
            nc.vector.tensor_scalar_mul(nlse[:, :], lse[:, :], -1.0)

            # ---- subtract lse, split across DVE and ACT, then store ----
            off = 0
            for c, ch in enumerate(chunks):
                sl = logits[0:1, off:off + ch]
                if c % 2 == 0:
                    nc.vector.tensor_scalar(
                        sl, sl, nlse[0:1, 0:1], None, op0=mybir.AluOpType.add
                    )
                else:
                    nc.scalar.activation(
                        sl, sl, mybir.ActivationFunctionType.Identity,
                        bias=nlse[0:1, 0:1],
                    )
                off += ch
            nc.gpsimd.dma_start(out=out_lp[0:1, 0:vpc], in_=logits[0:1, 0:vpc])

    nc.compile()  # bacc passes: wait splitting, reg alloc, event semaphores
    return nc


def prep_inputs(
    token, hidden, emb_table, W_ih, W_hh, b_ih, b_hh, W_out, b_out,
    ncores=NCORES, kt=KT, chunks=CHUNKS,
):
    """Host-side sharding: build the per-core input maps (pure numpy)."""
    emb = kt * 128
    vpc = sum(chunks)
    vpad = vpc * ncores
    esh = emb // ncores  # E-shard per core (= 128 when kt=8, ncores=8)
    assert esh * ncores == emb

    token = int(np.asarray(token).reshape(-1)[0])
    x_row = np.asarray(emb_table[token], np.float32).reshape(emb)
    h_row = np.asarray(hidden, np.float32).reshape(emb)
    W_ih = np.asarray(W_ih, np.float32)
    W_hh = np.asarray(W_hh, np.float32)
    b_ih = np.asarray(b_ih, np.float32)
    b_hh = np.asarray(b_hh, np.float32)
    W_out = np.asarray(W_out, np.float32)
    b_out = np.asarray(b_out, np.float32)
    vocab = W_out.shape[0]

    xh = np.empty((128, 2 * kt), np.float32)
    xh[:, 0::2] = x_row.reshape(kt, 128).T
    xh[:, 1::2] = h_row.reshape(kt, 128).T

    cst = np.array([[1, 1, 0, 1], [1, 0, 1, 0]], np.float32)

    Wp = np.zeros((vpad, emb), np.float32)
    Wp[:vocab] = W_out
    bp = np.full(vpad, BPAD, np.float32)
    bp[:vocab] = b_out

    in_maps = []
    for i in range(ncores):
        # GRU weight blocks, j = (gate*2 + mat)*kt + k
        blocks = np.empty((6 * kt, 128, 128), np.float32)
        for g in range(3):
            for m, W in enumerate((W_ih, W_hh)):
                rows = W[g * emb + i * esh:g * emb + (i + 1) * esh]  # [esh, emb]
                for k in range(kt):
                    j = (g * 2 + m) * kt + k
                    blk = np.zeros((128, 128), np.float32)
                    blk[:, :esh] = rows[:, k * 128:(k + 1) * 128].T
                    blocks[j] = blk
        wgru_i = np.ascontiguousarray(
            blocks.transpose(1, 0, 2).reshape(128, 6 * kt * 128)
        )

        bstk_i = np.zeros((2, 384), np.float32)
        for g in range(3):
            bstk_i[0, g * 128:g * 128 + esh] = b_ih[
                g * emb + i * esh:g * emb + (i + 1) * esh
            ]
            bstk_i[1, g * 128:g * 128 + esh] = b_hh[
                g * emb + i * esh:g * emb + (i + 1) * esh
            ]

        hcol_i = np.zeros((128, 1), np.float32)
        hcol_i[:esh, 0] = h_row[i * esh:(i + 1) * esh]

        shard = Wp[i * vpc:(i + 1) * vpc]            # [vpc, emb]
        shT = shard.T.reshape(kt, 128, vpc)          # [k, kappa, v]
        parts = []
        offv = 0
        for ch in chunks:
            blk = shT[:, :, offv:offv + ch]          # [k, kappa, ch]
            parts.append(blk.transpose(1, 0, 2).reshape(128, kt * ch))
            offv += ch
        wout_i = np.ascontiguousarray(np.concatenate(parts, axis=1))

        bout_i = bp[i * vpc:(i + 1) * vpc].reshape(1, vpc)

        in_maps.append(
            {
                "xh": xh,
                "hcol": hcol_i,
                "w_gru": wgru_i,
                "b_stack": bstk_i,
                "cst": cst,
                "w_out": wout_i,
                "b_out": np.ascontiguousarray(bout_i),
                "onesr": np.ones((1, 1), np.float32),
            }
        )
    return in_maps


def assemble_outputs(results, vocab=VOCAB, ncores=NCORES, chunks=CHUNKS, kt=KT):
    vpc = sum(chunks)
    lp = np.concatenate(
        [np.asarray(results[i]["out_logp"]).reshape(vpc) for i in range(ncores)]
    )[:vocab].reshape(1, vocab).astype(np.float32)
    h_new = np.asarray(results[0]["out_h"]).reshape(1, 1, kt * 128)
    return lp, h_new.astype(np.float32)


_NC_CACHE = {}


def _get_nc(mm_dtype=MM_DTYPE):
    if mm_dtype not in _NC_CACHE:
        _NC_CACHE[mm_dtype] = build_nc(mm_dtype=mm_dtype)
    return _NC_CACHE[mm_dtype]


def run(inputs, mm_dtype=MM_DTYPE, trace=False):
    nc = _get_nc(mm_dtype)
    in_maps = prep_inputs(
        inputs["token"], inputs["hidden"], inputs["emb_table"],
        inputs["W_ih"], inputs["W_hh"], inputs["b_ih"], inputs["b_hh"],
        inputs["W_out"], inputs["b_out"],
    )
    res = run_bass_kernel_spmd(
        nc, in_maps, core_ids=list(range(NCORES)), trace=trace
    )
    return res


def kernel(**inputs):
    res = run(inputs)
    return assemble_outputs(res.results)


# revision 17
# speedup vs baseline: 1.1503x; 1.1503x over previous
"""Trainium2 Bass kernel for a single-step DecoderRNN (embed+ReLU -> GRU -> logits -> log_softmax).

Sharding across 8 NeuronCores:
  - GRU weights are E-sharded: core i computes h_new[i*128:(i+1)*128] (one
    128-slice of each of the r/z/n gates), then an AllGather assembles the
    full h_new on every core.
  - W_out / b_out are vocab-sharded (row-parallel): core i computes logits
    for its padded 6400-row vocab shard, plus a local sum(exp(logits)).
    An AllGather of the 8 partial sums gives every core the global
    log-sum-exp, which is subtracted locally (no max-subtraction needed:
    logits are O(1) for this model so exp() cannot overflow).

The dominant cost is streaming the W_out shard (25.6 MB/core) from HBM, so
the kernel is one long chunk-major stream on the Sync DMA queue, while the
GRU + h_new AllGather run concurrently; a tiny barrier AllGather at kernel
start absorbs cross-core launch skew so the mid-kernel collective doesn't
stall the stream.
"""

import sys

sys.path.insert(0, "/opt/trn_rl_repo")

import numpy as np

from concourse import bacc, mybir, tile
from concourse.bass_utils import run_bass_kernel_spmd

F32 = mybir.dt.float32
F32R = mybir.dt.float32r

VOCAB = 50257
EMB = 1024
NCORES = 8
KT = EMB // 128           # 8 k-tiles over the embedding dim
CHUNKS = [512] * 12 + [256]
VPC = sum(CHUNKS)         # 6400 padded vocab rows per core
VPAD = VPC * NCORES       # 51200
BPAD = -87.0              # bias for padded vocab rows: exp(-87) ~= 0

# float32r streams the moving matmul operand at 1 cycle/row (vs 4 for
# float32, which walrus splits into two half-rate passes) at fp32-class
# accuracy (measured ~4e-6 rel on this net).
MM_DTYPE = "fp32r"


def build_nc(ncores=NCORES, kt=KT, chunks=CHUNKS, mm_dtype=MM_DTYPE):
    """Build the SPMD Bass program (identical on all cores; data differs)."""
    emb = kt * 128
    vpc = sum(chunks)
    assert ncores == kt, "E-shard per core must be exactly 128 rows"
    nc = bacc.Bacc(
        "TRN2", target_bir_lowering=False, debug=False, num_devices=ncores
    )

    # All matmul operands are declared float32r in fp32r mode: the BIR
    # verifier requires every producer of an fp32r-matmult operand to be
    # typed float32r, so the whole dataflow (DRAM input -> DMA -> SBUF tile
    # -> matmul) uses wdt. The h_new row-ification matmul stays plain fp32
    # (its lhsT is produced by a DVE op, which cannot emit float32r).
    wdt = F32R if mm_dtype == "fp32r" else F32

    def mk(ap):
        return ap.bitcast(wdt) if ap.dtype != wdt else ap

    # ---- kernel I/O (per core) ----
    # xh: interleaved grids, col 2k = relu(x)[k*128:(k+1)*128], col 2k+1 =
    #     h[k*128:(k+1)*128] (ReLU applied on host during input prep)
    xh_d = nc.dram_tensor("xh", [128, 2 * kt], wdt, kind="ExternalInput").ap()
    hcol_d = nc.dram_tensor("hcol", [128, 1], F32, kind="ExternalInput").ap()
    wgru_d = nc.dram_tensor(
        "w_gru", [128, 6 * kt * 128], wdt, kind="ExternalInput"
    ).ap()  # blocks j=(gate*2+mat)*kt+k at cols j*128..(j+1)*128
    bstk_d = nc.dram_tensor("b_stack", [2, 384], wdt, kind="ExternalInput").ap()
    cst_d = nc.dram_tensor("cst", [2, 6], wdt, kind="ExternalInput").ap()
    idm_d = nc.dram_tensor("idm", [128, 128], F32, kind="ExternalInput").ap()
    wout_d = nc.dram_tensor(
        "w_out", [128, kt * vpc], wdt, kind="ExternalInput"
    ).ap()
    bout_d = nc.dram_tensor("b_out", [1, vpc], wdt, kind="ExternalInput").ap()
    onesr_d = nc.dram_tensor("onesr", [1, 1], wdt, kind="ExternalInput").ap()

    out_lp = nc.dram_tensor("out_logp", [1, vpc], F32, kind="ExternalOutput").ap()
    out_h = nc.dram_tensor("out_h", [1, emb], F32, kind="ExternalOutput").ap()

    groups = [list(range(ncores))]

    with tile.TileContext(nc) as tc:
        with (
            tc.tile_pool(name="consts", bufs=1) as cpool,
            tc.tile_pool(name="wgru", bufs=1) as gpool,
            tc.tile_pool(name="wout", bufs=6) as wpool,
            tc.tile_pool(name="work", bufs=1) as spool,
            tc.tile_pool(name="esc", bufs=2) as epool,
            tc.tile_pool(name="gps", bufs=2, space="PSUM") as pgpool,
            tc.tile_pool(name="gpn", bufs=1, space="PSUM") as pnpool,
            tc.tile_pool(name="cps", bufs=5, space="PSUM") as pcpool,
            tc.tile_pool(name="dram", bufs=1, space="DRAM") as dpool,
        ):
            # ---- startup barrier: absorb cross-core launch skew here (it
            # overlaps the W_out stream) so the h_new AllGather sees aligned
            # cores instead of paying the skew on the critical path.
            bar_in = dpool.tile([1, 1], wdt, tag="bar_in")
            bar_out = dpool.tile([ncores, 1], wdt, tag="bar_out")
            nc.gpsimd.dma_start(out=bar_in[:, :], in_=onesr_d[:, :])
            nc.gpsimd.collective_compute(
                "AllGather",
                mybir.AluOpType.bypass,
                replica_groups=groups,
                ins=[bar_in.opt()],
                outs=[bar_out.opt()],
            )

            # ---- GRU weights first on the Sync queue, then the W_out
            # chunks; small loads go on the Scalar queue so trigger-issue
            # cost does not delay the streams.
            piece = kt * 128
            wg = []
            for q in range(6):
                t = gpool.tile([128, piece], wdt, tag=f"wg{q}")
                nc.sync.dma_start(
                    out=t[:, :], in_=wgru_d[:, q * piece:(q + 1) * piece]
                )
                wg.append(t)

            xh = cpool.tile([128, 2 * kt], wdt, tag="xh")
            nc.scalar.dma_start(out=xh[:, :], in_=xh_d[:, :])
            hcol = cpool.tile([128, 1], F32, tag="hcol")
            nc.scalar.dma_start(out=hcol[:, :], in_=hcol_d[:, :])
            bstk = cpool.tile([2, 384], wdt, tag="bstk")
            nc.scalar.dma_start(out=bstk[:, :], in_=bstk_d[:, :])
            cst = cpool.tile([2, 6], wdt, tag="cst")
            nc.scalar.dma_start(out=cst[:, :], in_=cst_d[:, :])
            idm = cpool.tile([128, 128], F32, tag="idm")
            nc.scalar.dma_start(out=idm[:, :], in_=idm_d[:, :])
            bout = cpool.tile([1, vpc], wdt, tag="bout")
            nc.scalar.dma_start(out=bout[:, :], in_=bout_d[:, :])
            onesr = cpool.tile([1, 1], wdt, tag="onesr")
            nc.scalar.dma_start(out=onesr[:, :], in_=onesr_d[:, :])

            # ---- GRU gate matvecs ----
            # fp32r matmuls need an even moving free dim, so every GRU mm
            # runs N=2 over an adjacent (x, h) column pair; the unwanted
            # column lands in a scratch psum column that is never read.
            # Gate psum layout [128, 4]: cols 0:2 accumulate W_ih@(x|h),
            # cols 2:4 accumulate W_hh@(x|h) -> wanted values at col 0
            # (W_ih@x) and col 3 (W_hh@h).
            # cst rhs patterns: G_sum=[[1,0],[1,0]] (b_ih+b_hh -> col 0),
            # G_ih=[[1,0],[0,0]], G_hh=[[0,0],[0,1]].
            g_sum = cst[0:2, 0:2]
            g_ih = cst[0:2, 2:4]
            g_hh = cst[0:2, 4:6]

            # start=True marks the ENTIRE 2KB psum bank pending-zero; the
            # first write to a pending byte overwrites (clearing the flag),
            # later writes accumulate. So exactly one start per psum tile:
            # the m=1 range's first matmul then auto-overwrites its own
            # still-pending bytes.
            def gate_mm(ps4, g):
                for m in range(2):
                    for k in range(kt):
                        nc.tensor.matmul(
                            ps4[:, 2 * m:2 * m + 2],
                            lhsT=wg[g * 2 + m][:, k * 128:(k + 1) * 128],
                            rhs=xh[:, 2 * k:2 * k + 2],
                            start=(m == 0 and k == 0),
                            stop=False,
                            skip_group_check=True,
                        )

            ps_r = pgpool.tile([128, 4], F32, tag="ps_rz")
            gate_mm(ps_r, 0)
            nc.tensor.matmul(
                ps_r[:, 0:2], lhsT=bstk[:, 0:128], rhs=g_sum,
                start=False, stop=True, skip_group_check=True,
            )
            ps_z = pgpool.tile([128, 4], F32, tag="ps_rz")
            gate_mm(ps_z, 1)
            nc.tensor.matmul(
                ps_z[:, 0:2], lhsT=bstk[:, 128:256], rhs=g_sum,
                start=False, stop=True, skip_group_check=True,
            )
            ps_n = pnpool.tile([128, 4], F32, tag="ps_n")
            gate_mm(ps_n, 2)
            nc.tensor.matmul(
                ps_n[:, 0:2], lhsT=bstk[:, 256:384], rhs=g_ih,
                start=False, stop=False, skip_group_check=True,
            )
            nc.tensor.matmul(
                ps_n[:, 2:4], lhsT=bstk[:, 256:384], rhs=g_hh,
                start=False, stop=True, skip_group_check=True,
            )

            # ---- GRU elementwise (all [128,1], partition-parallel) ----
            rpre = spool.tile([128, 1], F32, tag="rpre")
            nc.vector.tensor_reduce(
                rpre[:, :], ps_r[:, 0:4:3],
                axis=mybir.AxisListType.X, op=mybir.AluOpType.add,
            )
            r_sb = spool.tile([128, 1], F32, tag="r")
            nc.scalar.activation(
                r_sb[:, :], rpre[:, :], mybir.ActivationFunctionType.Sigmoid
            )
            zpre = spool.tile([128, 1], F32, tag="zpre")
            nc.vector.tensor_reduce(
                zpre[:, :], ps_z[:, 0:4:3],
                axis=mybir.AxisListType.X, op=mybir.AluOpType.add,
            )
            z_sb = spool.tile([128, 1], F32, tag="z")
            nc.scalar.activation(
                z_sb[:, :], zpre[:, :], mybir.ActivationFunctionType.Sigmoid
            )
            n2 = spool.tile([128, 2], F32, tag="n2")
            nc.vector.tensor_copy(n2[:, :], ps_n[:, 0:4:3])
            npre = spool.tile([128, 1], F32, tag="npre")
            # npre = (h_n * r) + i_n
            nc.vector.scalar_tensor_tensor(
                npre[:, :], n2[:, 1:2], r_sb[:, :], n2[:, 0:1],
                op0=mybir.AluOpType.mult, op1=mybir.AluOpType.add,
            )
            n_sb = spool.tile([128, 1], F32, tag="n")
            nc.scalar.activation(
                n_sb[:, :], npre[:, :], mybir.ActivationFunctionType.Tanh
            )
            d_sb = spool.tile([128, 1], F32, tag="d")
            nc.vector.tensor_sub(d_sb[:, :], hcol[:, :], n_sb[:, :])
            hnew = spool.tile([128, 1], F32, tag="hnew")
            # h_new = (h - n) * z + n
            nc.vector.scalar_tensor_tensor(
                hnew[:, :], d_sb[:, :], z_sb[:, :], n_sb[:, :],
                op0=mybir.AluOpType.mult, op1=mybir.AluOpType.add,
            )

            # ---- row-ify h_new (fast contiguous DMA to the collective):
            # [1,128] = hnew.T @ I_128 on the PE, then copy psum -> sbuf.
            ps_row = pnpool.tile([1, 128], F32, tag="ps_n")
            nc.tensor.matmul(
                ps_row[:, :], lhsT=hnew[:, :], rhs=idm[:, :],
                start=True, stop=True,
            )
            hrow = spool.tile([1, 128], F32, tag="hrow")
            nc.vector.tensor_copy(hrow[:, :], ps_row[:, :])

            # ---- AllGather h_new slices -> full h_new on every core ----
            cc_in = dpool.tile([1, 128], F32, tag="cc_in")
            cc_out = dpool.tile([ncores, 128], F32, tag="cc_out")
            nc.gpsimd.dma_start(out=cc_in[:, :], in_=hrow[:, :])
            nc.gpsimd.collective_compute(
                "AllGather",
                mybir.AluOpType.bypass,
                replica_groups=groups,
                ins=[cc_in.opt()],
                outs=[cc_out.opt()],
            )
            # full h_new, grid layout [128, kt] (col k = h_new[k*128:(k+1)*128])
            hg = spool.tile([128, kt], wdt, tag="hg")
            nc.gpsimd.dma_start(
                out=hg[:, 0:kt], in_=mk(cc_out[:, :].transpose([1, 0]))
            )
            # also emit the h_new kernel output (layouts match linearly)
            nc.gpsimd.dma_start(out=out_h[0:1, 0:emb], in_=cc_out[:, :])

            # ---- output projection: logits chunks + exp-sums ----
            logits = spool.tile([1, vpc], F32, tag="logits")
            sums = spool.tile([1, 16], F32, tag="sums")
            off = 0
            for c, ch in enumerate(chunks):
                wt = wpool.tile([128, kt * 512], wdt, tag="wt")
                nc.sync.dma_start(
                    out=wt[:, 0:kt * ch],
                    in_=wout_d[:, kt * off:kt * (off + ch)],
                )
                ps = pcpool.tile([1, 512], F32, tag="cps")
                for k in range(kt):
                    nc.tensor.matmul(
                        ps[0:1, 0:ch],
                        lhsT=hg[:, k:k + 1],
                        rhs=wt[:, k * ch:(k + 1) * ch],
                        start=(k == 0),
                        stop=False,
                    )
                # fold in b_out via a K=1 matmul with a ones stationary
                nc.tensor.matmul(
                    ps[0:1, 0:ch],
                    lhsT=onesr[0:1, 0:1],
                    rhs=bout[0:1, off:off + ch],
                    start=False,
                    stop=True,
                )
                nc.vector.tensor_copy(logits[0:1, off:off + ch], ps[0:1, 0:ch])
                esc = epool.tile([1, 512], F32, tag="esc")
                nc.scalar.activation(
                    esc[0:1, 0:ch],
                    ps[0:1, 0:ch],
                    mybir.ActivationFunctionType.Exp,
                    accum_out=sums[0:1, c:c + 1],
                )
                off += ch

            # ---- global log-sum-exp via AllGather of the 8 partial sums ----
            sloc = spool.tile([1, 1], F32, tag="sloc")
            nc.vector.tensor_reduce(
                sloc[:, :], sums[0:1, 0:len(chunks)],
                axis=mybir.AxisListType.X, op=mybir.AluOpType.add,
            )
            cc2_in = dpool.tile([1, 1], F32, tag="cc2_in")
            cc2_out = dpool.tile([ncores, 1], F32, tag="cc2_out")
            nc.gpsimd.dma_start(out=cc2_in[:, :], in_=sloc[:, :])
            nc.gpsimd.collective_compute(
                "AllGather",
                mybir.AluOpType.bypass,
                replica_groups=groups,
                ins=[cc2_in.opt()],
                outs=[cc2_out.opt()],
            )
            s8 = spool.tile([1, ncores], F32, tag="s8")
            nc.gpsimd.dma_start(out=s8[0:1, 0:ncores], in_=cc2_out[:, :])
            stot = spool.tile([1, 1], F32, tag="stot")
            nc.vector.tensor_reduce(
                stot[:, :], s8[0:1, 0:ncores],
                axis=mybir.AxisListType.X, op=mybir.AluOpType.add,
            )
            lse = spool.tile([1, 1], F32, tag="lse")
            nc.scalar.activation(
                lse[:, :], stot[:, :], mybir.ActivationFunctionType.Ln
            )
            nlse = spool.tile([1, 1], F32, tag="nlse")
            nc.vector.tensor_scalar_mul(nlse[:, :], lse[:, :], -1.0)

            # ---- subtract lse, split across DVE and ACT, then store ----
            off = 0
            for c, ch in enumerate(chunks):
                sl = logits[0:1, off:off + ch]
                if c % 2 == 0:
                    nc.vector.tensor_scalar(
                        sl, sl, nlse[0:1, 0:1], None, op0=mybir.AluOpType.add
                    )
                else:
                    nc.scalar.activation(
                        sl, sl, mybir.ActivationFunctionType.Identity,
                        bias=nlse[0:1, 0:1],
                    )
                off += ch
            nc.gpsimd.dma_start(out=out_lp[0:1, 0:vpc], in_=logits[0:1, 0:vpc])

    nc.compile()  # bacc passes: wait splitting, reg alloc, event semaphores
    return nc


def prep_inputs(
    token, hidden, emb_table, W_ih, W_hh, b_ih, b_hh, W_out, b_out,
    ncores=NCORES, kt=KT, chunks=CHUNKS,
):
    """Host-side sharding: build the per-core input maps (pure numpy)."""
    emb = kt * 128
    vpc = sum(chunks)
    vpad = vpc * ncores
    esh = emb // ncores  # E-shard per core (= 128 when kt=8, ncores=8)
    assert esh * ncores == emb

    token = int(np.asarray(token).reshape(-1)[0])
    x_row = np.maximum(
        np.asarray(emb_table[token], np.float32).reshape(emb), 0.0
    )
    h_row = np.asarray(hidden, np.float32).reshape(emb)
    W_ih = np.asarray(W_ih, np.float32)
    W_hh = np.asarray(W_hh, np.float32)
    b_ih = np.asarray(b_ih, np.float32)
    b_hh = np.asarray(b_hh, np.float32)
    W_out = np.asarray(W_out, np.float32)
    b_out = np.asarray(b_out, np.float32)
    vocab = W_out.shape[0]

    xh = np.empty((128, 2 * kt), np.float32)
    xh[:, 0::2] = x_row.reshape(kt, 128).T
    xh[:, 1::2] = h_row.reshape(kt, 128).T

    cst = np.array([[1, 0, 1, 0, 0, 0], [1, 0, 0, 0, 0, 1]], np.float32)
    idm = np.eye(128, dtype=np.float32)

    Wp = np.zeros((vpad, emb), np.float32)
    Wp[:vocab] = W_out
    bp = np.full(vpad, BPAD, np.float32)
    bp[:vocab] = b_out

    in_maps = []
    for i in range(ncores):
        # GRU weight blocks, j = (gate*2 + mat)*kt + k
        blocks = np.empty((6 * kt, 128, 128), np.float32)
        for g in range(3):
            for m, W in enumerate((W_ih, W_hh)):
                rows = W[g * emb + i * esh:g * emb + (i + 1) * esh]  # [esh, emb]
                for k in range(kt):
                    j = (g * 2 + m) * kt + k
                    blk = np.zeros((128, 128), np.float32)
                    blk[:, :esh] = rows[:, k * 128:(k + 1) * 128].T
                    blocks[j] = blk
        wgru_i = np.ascontiguousarray(
            blocks.transpose(1, 0, 2).reshape(128, 6 * kt * 128)
        )

        bstk_i = np.zeros((2, 384), np.float32)
        for g in range(3):
            bstk_i[0, g * 128:g * 128 + esh] = b_ih[
                g * emb + i * esh:g * emb + (i + 1) * esh
            ]
            bstk_i[1, g * 128:g * 128 + esh] = b_hh[
                g * emb + i * esh:g * emb + (i + 1) * esh
            ]

        hcol_i = np.zeros((128, 1), np.float32)
        hcol_i[:esh, 0] = h_row[i * esh:(i + 1) * esh]

        shard = Wp[i * vpc:(i + 1) * vpc]            # [vpc, emb]
        shT = shard.T.reshape(kt, 128, vpc)          # [k, kappa, v]
        parts = []
        offv = 0
        for ch in chunks:
            blk = shT[:, :, offv:offv + ch]          # [k, kappa, ch]
            parts.append(blk.transpose(1, 0, 2).reshape(128, kt * ch))
            offv += ch
        wout_i = np.ascontiguousarray(np.concatenate(parts, axis=1))

        bout_i = bp[i * vpc:(i + 1) * vpc].reshape(1, vpc)

        in_maps.append(
            {
                "xh": xh,
                "hcol": hcol_i,
                "w_gru": wgru_i,
                "b_stack": bstk_i,
                "cst": cst,
                "idm": idm,
                "w_out": wout_i,
                "b_out": np.ascontiguousarray(bout_i),
                "onesr": np.ones((1, 1), np.float32),
            }
        )
    return in_maps


def assemble_outputs(results, vocab=VOCAB, ncores=NCORES, chunks=CHUNKS, kt=KT):
    vpc = sum(chunks)
    lp = np.concatenate(
        [np.asarray(results[i]["out_logp"]).reshape(vpc) for i in range(ncores)]
    )[:vocab].reshape(1, vocab).astype(np.float32)
    h_new = np.asarray(results[0]["out_h"]).reshape(1, 1, kt * 128)
    return lp, h_new.astype(np.float32)


_NC_CACHE = {}


def _get_nc(mm_dtype=MM_DTYPE):
    if mm_dtype not in _NC_CACHE:
        _NC_CACHE[mm_dtype] = build_nc(mm_dtype=mm_dtype)
    return _NC_CACHE[mm_dtype]


def run(inputs, mm_dtype=MM_DTYPE, trace=False):
    nc = _get_nc(mm_dtype)
    in_maps = prep_inputs(
        inputs["token"], inputs["hidden"], inputs["emb_table"],
        inputs["W_ih"], inputs["W_hh"], inputs["b_ih"], inputs["b_hh"],
        inputs["W_out"], inputs["b_out"],
    )
    res = run_bass_kernel_spmd(
        nc, in_maps, core_ids=list(range(NCORES)), trace=trace
    )
    return res


def kernel(**inputs):
    res = run(inputs)
    return assemble_outputs(res.results)


# revision 19
# speedup vs baseline: 1.2558x; 1.0917x over previous
"""Trainium2 Bass kernel for a single-step DecoderRNN (embed+ReLU -> GRU -> logits -> log_softmax).

Sharding across 8 NeuronCores:
  - GRU weights are E-sharded: core i computes h_new[i*128:(i+1)*128] (one
    128-slice of each of the r/z/n gates), then an AllGather assembles the
    full h_new on every core.
  - W_out / b_out are vocab-sharded (row-parallel): core i computes logits
    for its padded 6400-row vocab shard, plus a local sum(exp(logits)).
    An AllGather of the 8 partial sums gives every core the global
    log-sum-exp, which is subtracted locally (no max-subtraction needed:
    logits are O(1) for this model so exp() cannot overflow).

The dominant cost is streaming the W_out shard (25.6 MB/core) from HBM, so
the kernel is one long chunk-major stream on the Sync DMA queue, while the
GRU + h_new AllGather run concurrently; a tiny barrier AllGather at kernel
start absorbs cross-core launch skew so the mid-kernel collective doesn't
stall the stream.
"""

import sys

sys.path.insert(0, "/opt/trn_rl_repo")

import numpy as np

from concourse import bacc, mybir, tile
from concourse.bass_utils import run_bass_kernel_spmd

F32 = mybir.dt.float32
F32R = mybir.dt.float32r
F16 = mybir.dt.float16

VOCAB = 50257
EMB = 1024
NCORES = 8
KT = EMB // 128           # 8 k-tiles over the embedding dim
CHUNKS = [512] * 12 + [256]
VPC = sum(CHUNKS)         # 6400 padded vocab rows per core
VPAD = VPC * NCORES       # 51200
BPAD = -87.0              # bias for padded vocab rows: exp(-87) ~= 0

# Matmul dataflow dtype. "fp16" halves the W_out stream (13 MB/core) and
# runs single-pass on the PE with FWL weight loads; absolute logits error
# ~3e-4 while log-softmax outputs are O(10), so relative error stays ~3e-5.
# "fp32r" keeps full fp32 inputs at 1 cycle/row (~5e-6 rel), "fp32" is the
# conservative 4-cycle/row fallback.
MM_DTYPE = "fp16"


def build_nc(ncores=NCORES, kt=KT, chunks=CHUNKS, mm_dtype=MM_DTYPE):
    """Build the SPMD Bass program (identical on all cores; data differs)."""
    emb = kt * 128
    vpc = sum(chunks)
    assert ncores == kt, "E-shard per core must be exactly 128 rows"
    nc = bacc.Bacc(
        "TRN2", target_bir_lowering=False, debug=False, num_devices=ncores
    )

    # All matmul operands are declared float32r in fp32r mode: the BIR
    # verifier requires every producer of an fp32r-matmult operand to be
    # typed float32r, so the whole dataflow (DRAM input -> DMA -> SBUF tile
    # -> matmul) uses wdt. The h_new row-ification matmul stays plain fp32
    # (its lhsT is produced by a DVE op, which cannot emit float32r).
    wdt = {"fp16": F16, "fp32r": F32R, "fp32": F32}[mm_dtype]

    def mk(ap):
        if mm_dtype == "fp32r" and ap.dtype != wdt:
            return ap.bitcast(wdt)
        return ap

    # ---- kernel I/O (per core) ----
    # xh: interleaved grids, col 2k = relu(x)[k*128:(k+1)*128], col 2k+1 =
    #     h[k*128:(k+1)*128] (ReLU applied on host during input prep)
    xh_d = nc.dram_tensor("xh", [128, 2 * kt], wdt, kind="ExternalInput").ap()
    hcol_d = nc.dram_tensor("hcol", [128, 1], F32, kind="ExternalInput").ap()
    wgru_d = nc.dram_tensor(
        "w_gru", [128, 6 * kt * 128], wdt, kind="ExternalInput"
    ).ap()  # blocks j=(gate*2+mat)*kt+k at cols j*128..(j+1)*128
    bstk_d = nc.dram_tensor("b_stack", [2, 384], wdt, kind="ExternalInput").ap()
    cst_d = nc.dram_tensor("cst", [2, 6], wdt, kind="ExternalInput").ap()
    idm_d = nc.dram_tensor("idm", [128, 128], wdt if mm_dtype == "fp16" else F32, kind="ExternalInput").ap()
    wout_d = nc.dram_tensor(
        "w_out", [128, kt * vpc], wdt, kind="ExternalInput"
    ).ap()
    bout_d = nc.dram_tensor("b_out", [1, vpc], wdt, kind="ExternalInput").ap()
    onesr_d = nc.dram_tensor("onesr", [1, 1], wdt, kind="ExternalInput").ap()

    out_lp = nc.dram_tensor("out_logp", [1, vpc], F32, kind="ExternalOutput").ap()
    out_h = nc.dram_tensor("out_h", [1, emb], F32, kind="ExternalOutput").ap()

    groups = [list(range(ncores))]

    with tile.TileContext(nc) as tc:
        with (
            tc.tile_pool(name="consts", bufs=1) as cpool,
            tc.tile_pool(name="wgru", bufs=1) as gpool,
            tc.tile_pool(name="wout", bufs=13 if mm_dtype == "fp16" else 6) as wpool,
            tc.tile_pool(name="work", bufs=1) as spool,
            tc.tile_pool(name="esc", bufs=2) as epool,
            tc.tile_pool(name="gps", bufs=2, space="PSUM") as pgpool,
            tc.tile_pool(name="gpn", bufs=1, space="PSUM") as pnpool,
            tc.tile_pool(name="cps", bufs=5, space="PSUM") as pcpool,
            tc.tile_pool(name="dram", bufs=1, space="DRAM") as dpool,
        ):
            # ---- startup barrier: absorb cross-core launch skew here (it
            # overlaps the W_out stream) so the h_new AllGather sees aligned
            # cores instead of paying the skew on the critical path.
            bar_in = dpool.tile([1, 1], wdt, tag="bar_in")
            bar_out = dpool.tile([ncores, 1], wdt, tag="bar_out")
            nc.gpsimd.dma_start(out=bar_in[:, :], in_=onesr_d[:, :])
            nc.gpsimd.collective_compute(
                "AllGather",
                mybir.AluOpType.bypass,
                replica_groups=groups,
                ins=[bar_in.opt()],
                outs=[bar_out.opt()],
            )

            # ---- GRU weights first on the Sync queue, then the W_out
            # chunks; small loads go on the Scalar queue so trigger-issue
            # cost does not delay the streams.
            piece = kt * 128
            wg = []
            for q in range(6):
                t = gpool.tile([128, piece], wdt, tag=f"wg{q}")
                nc.sync.dma_start(
                    out=t[:, :], in_=wgru_d[:, q * piece:(q + 1) * piece]
                )
                wg.append(t)

            xh = cpool.tile([128, 2 * kt], wdt, tag="xh")
            nc.scalar.dma_start(out=xh[:, :], in_=xh_d[:, :])
            hcol = cpool.tile([128, 1], F32, tag="hcol")
            nc.scalar.dma_start(out=hcol[:, :], in_=hcol_d[:, :])
            bstk = cpool.tile([2, 384], wdt, tag="bstk")
            nc.scalar.dma_start(out=bstk[:, :], in_=bstk_d[:, :])
            cst = cpool.tile([2, 6], wdt, tag="cst")
            nc.scalar.dma_start(out=cst[:, :], in_=cst_d[:, :])
            idm = cpool.tile([128, 128], wdt if mm_dtype == "fp16" else F32, tag="idm")
            nc.scalar.dma_start(out=idm[:, :], in_=idm_d[:, :])
            bout = cpool.tile([1, vpc], wdt, tag="bout")
            nc.scalar.dma_start(out=bout[:, :], in_=bout_d[:, :])
            onesr = cpool.tile([1, 1], wdt, tag="onesr")
            nc.scalar.dma_start(out=onesr[:, :], in_=onesr_d[:, :])

            # ---- GRU gate matvecs ----
            # fp32r matmuls need an even moving free dim, so every GRU mm
            # runs N=2 over an adjacent (x, h) column pair; the unwanted
            # column lands in a scratch psum column that is never read.
            # Gate psum layout [128, 4]: cols 0:2 accumulate W_ih@(x|h),
            # cols 2:4 accumulate W_hh@(x|h) -> wanted values at col 0
            # (W_ih@x) and col 3 (W_hh@h).
            # cst rhs patterns: G_sum=[[1,0],[1,0]] (b_ih+b_hh -> col 0),
            # G_ih=[[1,0],[0,0]], G_hh=[[0,0],[0,1]].
            g_sum = cst[0:2, 0:2]
            g_ih = cst[0:2, 2:4]
            g_hh = cst[0:2, 4:6]

            # start=True marks the ENTIRE 2KB psum bank pending-zero; the
            # first write to a pending byte overwrites (clearing the flag),
            # later writes accumulate. So exactly one start per psum tile:
            # the m=1 range's first matmul then auto-overwrites its own
            # still-pending bytes.
            def gate_mm(ps4, g):
                for m in range(2):
                    for k in range(kt):
                        nc.tensor.matmul(
                            ps4[:, 2 * m:2 * m + 2],
                            lhsT=wg[g * 2 + m][:, k * 128:(k + 1) * 128],
                            rhs=xh[:, 2 * k:2 * k + 2],
                            start=(m == 0 and k == 0),
                            stop=False,
                            skip_group_check=True,
                        )

            ps_r = pgpool.tile([128, 4], F32, tag="ps_rz")
            gate_mm(ps_r, 0)
            nc.tensor.matmul(
                ps_r[:, 0:2], lhsT=bstk[:, 0:128], rhs=g_sum,
                start=False, stop=True, skip_group_check=True,
            )
            ps_z = pgpool.tile([128, 4], F32, tag="ps_rz")
            gate_mm(ps_z, 1)
            nc.tensor.matmul(
                ps_z[:, 0:2], lhsT=bstk[:, 128:256], rhs=g_sum,
                start=False, stop=True, skip_group_check=True,
            )
            ps_n = pnpool.tile([128, 4], F32, tag="ps_n")
            gate_mm(ps_n, 2)
            nc.tensor.matmul(
                ps_n[:, 0:2], lhsT=bstk[:, 256:384], rhs=g_ih,
                start=False, stop=False, skip_group_check=True,
            )
            nc.tensor.matmul(
                ps_n[:, 2:4], lhsT=bstk[:, 256:384], rhs=g_hh,
                start=False, stop=True, skip_group_check=True,
            )

            # ---- GRU elementwise (all [128,1], partition-parallel) ----
            rpre = spool.tile([128, 1], F32, tag="rpre")
            nc.vector.tensor_reduce(
                rpre[:, :], ps_r[:, 0:4:3],
                axis=mybir.AxisListType.X, op=mybir.AluOpType.add,
            )
            r_sb = spool.tile([128, 1], F32, tag="r")
            nc.scalar.activation(
                r_sb[:, :], rpre[:, :], mybir.ActivationFunctionType.Sigmoid
            )
            zpre = spool.tile([128, 1], F32, tag="zpre")
            nc.vector.tensor_reduce(
                zpre[:, :], ps_z[:, 0:4:3],
                axis=mybir.AxisListType.X, op=mybir.AluOpType.add,
            )
            z_sb = spool.tile([128, 1], F32, tag="z")
            nc.scalar.activation(
                z_sb[:, :], zpre[:, :], mybir.ActivationFunctionType.Sigmoid
            )
            n2 = spool.tile([128, 2], F32, tag="n2")
            nc.vector.tensor_copy(n2[:, :], ps_n[:, 0:4:3])
            npre = spool.tile([128, 1], F32, tag="npre")
            # npre = (h_n * r) + i_n
            nc.vector.scalar_tensor_tensor(
                npre[:, :], n2[:, 1:2], r_sb[:, :], n2[:, 0:1],
                op0=mybir.AluOpType.mult, op1=mybir.AluOpType.add,
            )
            n_sb = spool.tile([128, 1], F32, tag="n")
            nc.scalar.activation(
                n_sb[:, :], npre[:, :], mybir.ActivationFunctionType.Tanh
            )
            d_sb = spool.tile([128, 1], F32, tag="d")
            nc.vector.tensor_sub(d_sb[:, :], hcol[:, :], n_sb[:, :])
            hnew = spool.tile([128, 1], F32, tag="hnew")
            # h_new = (h - n) * z + n
            nc.vector.scalar_tensor_tensor(
                hnew[:, :], d_sb[:, :], z_sb[:, :], n_sb[:, :],
                op0=mybir.AluOpType.mult, op1=mybir.AluOpType.add,
            )

            # ---- row-ify h_new (fast contiguous DMA to the collective):
            # [1,128] = hnew.T @ I_128 on the PE, then copy psum -> sbuf.
            # In fp16 mode the PE stream must stay fp16-only: an FP32 matmul
            # between FWL (fp16) weight loads can hang the PE (the compiler's
            # LastMatmultFP32HI guard does not cover the reorder window).
            if mm_dtype == "fp16":
                hnew_mm = spool.tile([128, 1], F16, tag="hnew16")
                nc.vector.tensor_copy(hnew_mm[:, :], hnew[:, :])
            else:
                hnew_mm = hnew
            ps_row = pnpool.tile([1, 128], F32, tag="ps_n")
            nc.tensor.matmul(
                ps_row[:, :], lhsT=hnew_mm[:, :], rhs=idm[:, :],
                start=True, stop=True,
            )
            hrow = spool.tile([1, 128], F32, tag="hrow")
            nc.vector.tensor_copy(hrow[:, :], ps_row[:, :])

            # ---- AllGather h_new slices -> full h_new on every core ----
            cc_in = dpool.tile([1, 128], F32, tag="cc_in")
            cc_out = dpool.tile([ncores, 128], F32, tag="cc_out")
            nc.gpsimd.dma_start(out=cc_in[:, :], in_=hrow[:, :])
            nc.gpsimd.collective_compute(
                "AllGather",
                mybir.AluOpType.bypass,
                replica_groups=groups,
                ins=[cc_in.opt()],
                outs=[cc_out.opt()],
            )
            # full h_new, grid layout [128, kt] (col k = h_new[k*128:(k+1)*128])
            if mm_dtype == "fp16":
                hg32 = spool.tile([128, kt], F32, tag="hg32")
                nc.gpsimd.dma_start(
                    out=hg32[:, 0:kt], in_=cc_out[:, :].transpose([1, 0])
                )
                hg = spool.tile([128, kt], F16, tag="hg")
                nc.vector.tensor_copy(hg[:, :], hg32[:, :])
            else:
                hg = spool.tile([128, kt], wdt, tag="hg")
                nc.gpsimd.dma_start(
                    out=hg[:, 0:kt], in_=mk(cc_out[:, :].transpose([1, 0]))
                )
            # also emit the h_new kernel output (layouts match linearly)
            nc.gpsimd.dma_start(out=out_h[0:1, 0:emb], in_=cc_out[:, :])

            # ---- output projection: logits chunks + exp-sums ----
            logits = spool.tile([1, vpc], F32, tag="logits")
            sums = spool.tile([1, 16], F32, tag="sums")
            off = 0
            for c, ch in enumerate(chunks):
                wt = wpool.tile([128, kt * 512], wdt, tag="wt")
                nc.sync.dma_start(
                    out=wt[:, 0:kt * ch],
                    in_=wout_d[:, kt * off:kt * (off + ch)],
                )
                ps = pcpool.tile([1, 512], F32, tag="cps")
                for k in range(kt):
                    nc.tensor.matmul(
                        ps[0:1, 0:ch],
                        lhsT=hg[:, k:k + 1],
                        rhs=wt[:, k * ch:(k + 1) * ch],
                        start=(k == 0),
                        stop=False,
                    )
                # fold in b_out via a K=1 matmul with a ones stationary
                nc.tensor.matmul(
                    ps[0:1, 0:ch],
                    lhsT=onesr[0:1, 0:1],
                    rhs=bout[0:1, off:off + ch],
                    start=False,
                    stop=True,
                )
                nc.vector.tensor_copy(logits[0:1, off:off + ch], ps[0:1, 0:ch])
                esc = epool.tile([1, 512], F32, tag="esc")
                nc.scalar.activation(
                    esc[0:1, 0:ch],
                    ps[0:1, 0:ch],
                    mybir.ActivationFunctionType.Exp,
                    accum_out=sums[0:1, c:c + 1],
                )
                off += ch

            # ---- global log-sum-exp via AllGather of the 8 partial sums ----
            sloc = spool.tile([1, 1], F32, tag="sloc")
            nc.vector.tensor_reduce(
                sloc[:, :], sums[0:1, 0:len(chunks)],
                axis=mybir.AxisListType.X, op=mybir.AluOpType.add,
            )
            cc2_in = dpool.tile([1, 1], F32, tag="cc2_in")
            cc2_out = dpool.tile([ncores, 1], F32, tag="cc2_out")
            nc.gpsimd.dma_start(out=cc2_in[:, :], in_=sloc[:, :])
            nc.gpsimd.collective_compute(
                "AllGather",
                mybir.AluOpType.bypass,
                replica_groups=groups,
                ins=[cc2_in.opt()],
                outs=[cc2_out.opt()],
            )
            s8 = spool.tile([1, ncores], F32, tag="s8")
            nc.gpsimd.dma_start(out=s8[0:1, 0:ncores], in_=cc2_out[:, :])
            stot = spool.tile([1, 1], F32, tag="stot")
            nc.vector.tensor_reduce(
                stot[:, :], s8[0:1, 0:ncores],
                axis=mybir.AxisListType.X, op=mybir.AluOpType.add,
            )
            lse = spool.tile([1, 1], F32, tag="lse")
            nc.scalar.activation(
                lse[:, :], stot[:, :], mybir.ActivationFunctionType.Ln
            )
            nlse = spool.tile([1, 1], F32, tag="nlse")
            nc.vector.tensor_scalar_mul(nlse[:, :], lse[:, :], -1.0)

            # ---- subtract lse, split across DVE and ACT, then store ----
            off = 0
            for c, ch in enumerate(chunks):
                sl = logits[0:1, off:off + ch]
                if c % 2 == 0:
                    nc.vector.tensor_scalar(
                        sl, sl, nlse[0:1, 0:1], None, op0=mybir.AluOpType.add
                    )
                else:
                    nc.scalar.activation(
                        sl, sl, mybir.ActivationFunctionType.Identity,
                        bias=nlse[0:1, 0:1],
                    )
                off += ch
            nc.gpsimd.dma_start(out=out_lp[0:1, 0:vpc], in_=logits[0:1, 0:vpc])

    nc.compile()  # bacc passes: wait splitting, reg alloc, event semaphores
    return nc


def prep_inputs(
    token, hidden, emb_table, W_ih, W_hh, b_ih, b_hh, W_out, b_out,
    ncores=NCORES, kt=KT, chunks=CHUNKS, mm_dtype=MM_DTYPE,
):
    """Host-side sharding: build the per-core input maps (pure numpy)."""
    emb = kt * 128
    vpc = sum(chunks)
    vpad = vpc * ncores
    esh = emb // ncores  # E-shard per core (= 128 when kt=8, ncores=8)
    assert esh * ncores == emb

    token = int(np.asarray(token).reshape(-1)[0])
    x_row = np.maximum(
        np.asarray(emb_table[token], np.float32).reshape(emb), 0.0
    )
    h_row = np.asarray(hidden, np.float32).reshape(emb)
    W_ih = np.asarray(W_ih, np.float32)
    W_hh = np.asarray(W_hh, np.float32)
    b_ih = np.asarray(b_ih, np.float32)
    b_hh = np.asarray(b_hh, np.float32)
    W_out = np.asarray(W_out, np.float32)
    b_out = np.asarray(b_out, np.float32)
    vocab = W_out.shape[0]

    xh = np.empty((128, 2 * kt), np.float32)
    xh[:, 0::2] = x_row.reshape(kt, 128).T
    xh[:, 1::2] = h_row.reshape(kt, 128).T

    cst = np.array([[1, 0, 1, 0, 0, 0], [1, 0, 0, 0, 0, 1]], np.float32)
    idm = np.eye(128, dtype=np.float32)

    Wp = np.zeros((vpad, emb), np.float32)
    Wp[:vocab] = W_out
    bp = np.full(vpad, BPAD, np.float32)
    bp[:vocab] = b_out

    mdt = np.float16 if mm_dtype == "fp16" else np.float32
    in_maps = []
    for i in range(ncores):
        # GRU weight blocks, j = (gate*2 + mat)*kt + k
        blocks = np.empty((6 * kt, 128, 128), np.float32)
        for g in range(3):
            for m, W in enumerate((W_ih, W_hh)):
                rows = W[g * emb + i * esh:g * emb + (i + 1) * esh]  # [esh, emb]
                for k in range(kt):
                    j = (g * 2 + m) * kt + k
                    blk = np.zeros((128, 128), np.float32)
                    blk[:, :esh] = rows[:, k * 128:(k + 1) * 128].T
                    blocks[j] = blk
        wgru_i = np.ascontiguousarray(
            blocks.transpose(1, 0, 2).reshape(128, 6 * kt * 128)
        )

        bstk_i = np.zeros((2, 384), np.float32)
        for g in range(3):
            bstk_i[0, g * 128:g * 128 + esh] = b_ih[
                g * emb + i * esh:g * emb + (i + 1) * esh
            ]
            bstk_i[1, g * 128:g * 128 + esh] = b_hh[
                g * emb + i * esh:g * emb + (i + 1) * esh
            ]

        hcol_i = np.zeros((128, 1), np.float32)
        hcol_i[:esh, 0] = h_row[i * esh:(i + 1) * esh]

        shard = Wp[i * vpc:(i + 1) * vpc]            # [vpc, emb]
        shT = shard.T.reshape(kt, 128, vpc)          # [k, kappa, v]
        parts = []
        offv = 0
        for ch in chunks:
            blk = shT[:, :, offv:offv + ch]          # [k, kappa, ch]
            parts.append(blk.transpose(1, 0, 2).reshape(128, kt * ch))
            offv += ch
        wout_i = np.ascontiguousarray(np.concatenate(parts, axis=1))

        bout_i = bp[i * vpc:(i + 1) * vpc].reshape(1, vpc)

        in_maps.append(
            {
                "xh": xh.astype(mdt),
                "hcol": hcol_i,
                "w_gru": wgru_i.astype(mdt),
                "b_stack": bstk_i.astype(mdt),
                "cst": cst.astype(mdt),
                "idm": idm.astype(mdt) if mm_dtype == "fp16" else idm,
                "w_out": wout_i.astype(mdt),
                "b_out": np.ascontiguousarray(bout_i).astype(mdt),
                "onesr": np.ones((1, 1), mdt),
            }
        )
    return in_maps


def assemble_outputs(results, vocab=VOCAB, ncores=NCORES, chunks=CHUNKS, kt=KT):
    vpc = sum(chunks)
    lp = np.concatenate(
        [np.asarray(results[i]["out_logp"]).reshape(vpc) for i in range(ncores)]
    )[:vocab].reshape(1, vocab).astype(np.float32)
    h_new = np.asarray(results[0]["out_h"]).reshape(1, 1, kt * 128)
    return lp, h_new.astype(np.float32)


_NC_CACHE = {}


def _get_nc(mm_dtype=MM_DTYPE):
    if mm_dtype not in _NC_CACHE:
        _NC_CACHE[mm_dtype] = build_nc(mm_dtype=mm_dtype)
    return _NC_CACHE[mm_dtype]


def run(inputs, mm_dtype=MM_DTYPE, trace=False):
    nc = _get_nc(mm_dtype)
    in_maps = prep_inputs(
        inputs["token"], inputs["hidden"], inputs["emb_table"],
        inputs["W_ih"], inputs["W_hh"], inputs["b_ih"], inputs["b_hh"],
        inputs["W_out"], inputs["b_out"], mm_dtype=mm_dtype,
    )
    res = run_bass_kernel_spmd(
        nc, in_maps, core_ids=list(range(NCORES)), trace=trace
    )
    return res


def kernel(**inputs):
    res = run(inputs)
    return assemble_outputs(res.results)
